# revision 16
# baseline (speedup 1.0000x reference)
"""AttentionPooling (segment softmax + weighted pooling) on 8 TRN2 NeuronCores.

Strategy (single pass over x):
  - Nodes are split evenly across 8 cores (uniform SPMD program; per-core data
    differs, program is identical). Each core's node range is processed in
    fixed-size "chunks" of T_CHUNK 128-node tiles.
  - Score path: x is uploaded in a pair-transposed bf16 layout (xT), so the
    gate MLP runs as plain matmuls with no on-chip transposes:
        hT = tanh(W1^T @ xT + b1)   (PE, bf16, fp32 accum; ACT tanh)
        s  = W2^T @ hT              (PE) -> per-pair PSUM rows
    Scores are bulk-transposed once per chunk (PE transpose of the [pairs,256]
    score block) and exponentiated (ACT) into per-node column layout.
  - Softmax max-subtraction is skipped: |s| <= sqrt(128)*||W2|| is bounded
    because tanh output is in [-1,1], so exp never overflows in fp32.
  - Pooling: for each 128-node tile, a one-hot matrix over a 128-segment
    window (exoh[i, g] = ex_i * (batch_local[i] == g)) is built on DVE in one
    tensor_scalar op from a constant iota tile, then a single PE matmul
    accumulates  U[window] += exoh^T @ x  into PSUM across the whole chunk.
  - Host combines the per-chunk 128-segment windows into the global (B, H)
    unnormalized sums, normalizes by Z (bincount of exp-scores), and forms the
    per-node weights ex/Z[batch].

Everything data-dependent (chunk window bases, exclusions) lives in the host
pre/post-processing; the device program depends only on (n_chunks, T_chunk).
"""

import math
import os
import time
from contextlib import ExitStack

import ml_dtypes
import numpy as np

import concourse.bass as bass
import concourse.mybir as mybir
import concourse.tile as tile
from concourse import bacc
from concourse.bass_utils import run_bass_kernel_spmd

NCORES = 8
P = 128          # partitions / nodes per tile
H = 256          # feature dim
NUM_GRAPHS = 4096
WIN = 128        # segment window width per chunk

BF16 = ml_dtypes.bfloat16

_prog_cache: dict = {}
_last_results = {}


def _build_program(n_chunks: int, t_chunk: int, bench_iters: int = 1):
    """Build the (per-core identical) Bass/Tile program.

    bench_iters > 1 wraps the whole body in an on-device For_i loop that
    recomputes identical outputs; used only for timing (idempotent body).
    """
    key = (n_chunks, t_chunk, bench_iters)
    if key in _prog_cache:
        return _prog_cache[key]

    assert t_chunk % 2 == 0
    tp = t_chunk // 2               # pairs per chunk
    assert tp <= 64
    n_pairs = n_chunks * tp

    bf = mybir.dt.bfloat16
    f32 = mybir.dt.float32
    TANH = mybir.ActivationFunctionType.Tanh
    EXP = mybir.ActivationFunctionType.Exp
    COPY = mybir.ActivationFunctionType.Copy
    EQ = mybir.AluOpType.is_equal
    MUL = mybir.AluOpType.mult

    nc = bacc.Bacc("TRN2", target_bir_lowering=False, debug=False,
                   num_devices=NCORES)

    xp_d = nc.dram_tensor("xp", [n_chunks * t_chunk, P, H], bf,
                          kind="ExternalInput").ap()
    xt_d = nc.dram_tensor("xt", [n_pairs, 2, P, 2 * P], bf,
                          kind="ExternalInput").ap()
    bl_d = nc.dram_tensor("bl", [n_chunks, P, t_chunk], f32,
                          kind="ExternalInput").ap()
    w1_d = nc.dram_tensor("w1", [2, P, P], bf, kind="ExternalInput").ap()
    w2_d = nc.dram_tensor("w2", [P, 1], bf, kind="ExternalInput").ap()
    b1_d = nc.dram_tensor("b1c", [P, 1], f32, kind="ExternalInput").ap()
    b2_d = nc.dram_tensor("b2c", [P, 1], f32, kind="ExternalInput").ap()
    io_d = nc.dram_tensor("iota", [P, P], bf, kind="ExternalInput").ap()

    ex_d = nc.dram_tensor("ex_out", [n_chunks, P, t_chunk], f32,
                          kind="ExternalOutput").ap()
    u_d = nc.dram_tensor("u_out", [n_chunks, P, H], f32,
                         kind="ExternalOutput").ap()

    with tile.TileContext(nc) as tc, ExitStack() as ctx:
        psum = bass.MemorySpace.PSUM
        cpool = ctx.enter_context(tc.tile_pool(name="consts", bufs=1))
        xt_p = ctx.enter_context(tc.tile_pool(name="xtp", bufs=3))
        xp_p = ctx.enter_context(tc.tile_pool(name="xpp", bufs=3))
        ht_p = ctx.enter_context(tc.tile_pool(name="htp", bufs=8))
        exs_p = ctx.enter_context(tc.tile_pool(name="exsp", bufs=2))
        eo_p = ctx.enter_context(tc.tile_pool(name="eop", bufs=10))
        bl_p = ctx.enter_context(tc.tile_pool(name="blp", bufs=2))
        usb_p = ctx.enter_context(tc.tile_pool(name="usbp", bufs=2))

        hps_p = ctx.enter_context(tc.tile_pool(name="hps", bufs=4, space=psum))
        sps_p = ctx.enter_context(tc.tile_pool(name="sps", bufs=2, space=psum))
        ups_p = ctx.enter_context(tc.tile_pool(name="ups", bufs=2, space=psum))

        # constants
        w1t = cpool.tile([P, 2 * P], bf, name="w1t")
        nc.sync.dma_start(w1t[:, 0:P], w1_d[0])
        nc.sync.dma_start(w1t[:, P:2 * P], w1_d[1])
        w2t = cpool.tile([P, 1], bf, name="w2t")
        nc.sync.dma_start(w2t[:], w2_d)
        b1t = cpool.tile([P, 1], f32, name="b1t")
        nc.sync.dma_start(b1t[:], b1_d)
        b2t = cpool.tile([P, 1], f32, name="b2t")
        nc.sync.dma_start(b2t[:], b2_d)
        iot = cpool.tile([P, P], bf, name="iot")
        nc.sync.dma_start(iot[:], io_d)

        # DMA batching: group xt pair-loads and xp tile-loads into ~1MB
        # transfers (per-dma_start fixed cost on SP.SEQ/HWDGE is ~650ns).
        def _groups(n, pref):
            g = min(pref, n)
            out = []
            i = 0
            while i < n:
                out.append((i, min(g, n - i)))
                i += g
            return out

        xt_groups = _groups(tp, 7)        # groups of pairs
        xp_groups = _groups(t_chunk, 14)  # groups of tiles

        loop_cm = (tc.For_i(0, bench_iters, 1,
                            hint_engines=(mybir.EngineType.PE,
                                          mybir.EngineType.DVE,
                                          mybir.EngineType.Activation,
                                          mybir.EngineType.SP))
                   if bench_iters > 1 else None)
        if loop_cm is not None:
            loop_cm.__enter__()
        for c in range(n_chunks):
            blt = bl_p.tile([P, t_chunk], f32, name="blt", tag="blt")
            nc.sync.dma_start(blt[:], bl_d[c])

            # ---- score path: one pair (256 nodes) at a time ----
            # s columns accumulate into one PSUM bank region [128, t_chunk];
            # each column is written exactly once inside one long
            # start..stop group (pending-zero semantics make the first
            # write of each column an overwrite).
            s_ps = sps_p.tile([P, t_chunk], f32, name="s_ps", tag="s_ps")
            for g0, gn in xt_groups:
                xtg = xt_p.tile([P, gn * 2 * 2 * P], bf, name="xtg", tag="xt")
                src = xt_d[c * tp + g0:c * tp + g0 + gn]
                nc.sync.dma_start(xtg[:], src.rearrange("g h p c -> p g h c"))
                for j in range(gn):
                    pi = g0 + j
                    xt0 = xtg[:, j * 4 * P:j * 4 * P + 2 * P]
                    xt1 = xtg[:, j * 4 * P + 2 * P:(j + 1) * 4 * P]
                    h_ps = hps_p.tile([P, 2 * P], f32, name="h_ps", tag="h_ps")
                    nc.tensor.matmul(h_ps[:], w1t[:, 0:P], xt0,
                                     start=True, stop=False)
                    nc.tensor.matmul(h_ps[:], w1t[:, P:2 * P], xt1,
                                     start=False, stop=True)
                    ht = ht_p.tile([P, 2 * P], bf, name="ht", tag="ht")
                    nc.scalar.activation(ht[:], h_ps[:], TANH,
                                         bias=b1t[:, 0:1], scale=1.0)
                    t0c = 2 * pi
                    nc.tensor.matmul(s_ps[:, t0c:t0c + 1], ht[:, 0:P], w2t[:],
                                     start=(pi == 0), stop=False)
                    nc.tensor.matmul(s_ps[:, t0c + 1:t0c + 2], ht[:, P:2 * P],
                                     w2t[:], start=False, stop=(pi == tp - 1))

            # ---- batched exp over the whole chunk ----
            exs = exs_p.tile([P, t_chunk], f32, name="exs", tag="exs")
            nc.scalar.activation(exs[:], s_ps[:], EXP, bias=b2t[:, 0:1],
                                 scale=1.0)
            nc.sync.dma_start(ex_d[c], exs[:])

            # ---- pooling: one 128-node tile at a time ----
            u_ps = ups_p.tile([P, H], f32, name="u_ps", tag="u_ps")
            for g0, gn in xp_groups:
                xpg = xp_p.tile([P, gn * H], bf, name="xpg", tag="xpt")
                src = xp_d[c * t_chunk + g0:c * t_chunk + g0 + gn]
                nc.sync.dma_start(xpg[:], src.rearrange("t p c -> p t c"))
                for j in range(gn):
                    t = g0 + j
                    eo = eo_p.tile([P, P], bf, name="eo", tag="eo")
                    nc.vector.tensor_scalar(eo[:], iot[:], blt[:, t:t + 1],
                                            exs[:, t:t + 1],
                                            op0=EQ, op1=MUL)
                    nc.tensor.matmul(u_ps[:], eo[:],
                                     xpg[:, j * H:(j + 1) * H],
                                     start=(t == 0),
                                     stop=(t == t_chunk - 1))
            u_sb = usb_p.tile([P, H], f32, name="u_sb", tag="u_sb")
            nc.vector.tensor_copy(u_sb[:], u_ps[:])
            nc.sync.dma_start(u_d[c], u_sb[:])
        if loop_cm is not None:
            loop_cm.__exit__(None, None, None)

    nc.compile()
    _prog_cache[key] = nc
    return nc


def _plan(n_nodes: int, batch: np.ndarray):
    """Pick (n_chunks, t_chunk) so chunk segment spans fit in WIN."""
    for t_chunk in (98, 96, 88, 80, 72, 64, 56, 48, 40, 32, 24, 16, 8, 4, 2):
        chunk_nodes = t_chunk * P
        per_core = math.ceil(n_nodes / (NCORES * chunk_nodes))
        total = NCORES * per_core * chunk_nodes
        # chunk start node indices (clipped into valid range)
        starts = np.arange(0, total, chunk_nodes)
        starts = starts[starts < n_nodes]
        ends = np.minimum(starts + chunk_nodes, n_nodes) - 1
        span = batch[ends] - batch[starts]
        if span.max(initial=0) < WIN:
            return per_core, t_chunk, total
    # fallback: smallest chunk; exclusions handled on host anyway
    t_chunk = 2
    chunk_nodes = t_chunk * P
    per_core = math.ceil(n_nodes / (NCORES * chunk_nodes))
    return per_core, t_chunk, NCORES * per_core * chunk_nodes


def kernel(x, batch, W1, b1, W2, b2):
    x = np.asarray(x, dtype=np.float32)
    batch = np.asarray(batch).astype(np.int64)
    W1 = np.asarray(W1, dtype=np.float32)
    b1 = np.asarray(b1, dtype=np.float32)
    W2 = np.asarray(W2, dtype=np.float32)
    b2 = np.asarray(b2, dtype=np.float32)

    n, h = x.shape
    assert h == H

    n_chunks, t_chunk, total = _plan(n, batch)
    tp = t_chunk // 2
    chunk_nodes = t_chunk * P
    core_nodes = n_chunks * chunk_nodes

    nc = _build_program(n_chunks, t_chunk)

    # ---------------- host-side input prep ----------------
    xb = np.zeros((total, H), dtype=BF16)
    xb[:n] = x.astype(BF16)
    batch_pad = np.full(total, -1, dtype=np.int64)
    batch_pad[:n] = batch

    # chunk window bases (global over all cores)
    n_chunks_total = NCORES * n_chunks
    first_idx = np.arange(n_chunks_total) * chunk_nodes
    valid = first_idx < n
    bases = np.zeros(n_chunks_total, dtype=np.int64)
    bases[valid] = batch[np.minimum(first_idx[valid], n - 1)]

    # batch-local window offsets
    bl = batch_pad.reshape(n_chunks_total, chunk_nodes) - bases[:, None]
    pad_mask = batch_pad.reshape(n_chunks_total, chunk_nodes) < 0
    excl_mask = (bl >= WIN) & ~pad_mask
    bl[pad_mask | excl_mask] = -1
    excl_idx = np.nonzero((excl_mask.reshape(-1)) & (np.arange(total) < n))[0]

    # [n_chunks_total, chunk_nodes] -> per core [n_chunks, P, t_chunk]
    bl_f = bl.astype(np.float32).reshape(NCORES, n_chunks, t_chunk, P)
    bl_f = np.ascontiguousarray(bl_f.transpose(0, 1, 3, 2))

    w1b = np.ascontiguousarray(W1.astype(BF16).reshape(2, P, P))
    w2b = np.ascontiguousarray(W2.astype(BF16).reshape(P, 1))
    b1c = np.ascontiguousarray(b1.reshape(P, 1))
    b2c = np.full((P, 1), float(np.asarray(b2).reshape(-1)[0]), np.float32)
    iota = np.ascontiguousarray(
        np.broadcast_to(np.arange(P, dtype=np.float32), (P, P))).astype(BF16)

    in_maps = []
    for k in range(NCORES):
        sl = xb[k * core_nodes:(k + 1) * core_nodes]
        xp_k = sl.reshape(n_chunks * t_chunk, P, H)
        xt_k = np.ascontiguousarray(
            sl.reshape(n_chunks * tp, 2 * P, H).transpose(0, 2, 1)
        ).reshape(n_chunks * tp, 2, P, 2 * P)
        in_maps.append({
            "xp": xp_k, "xt": xt_k, "bl": bl_f[k],
            "w1": w1b, "w2": w2b, "b1c": b1c, "b2c": b2c,
            "iota": iota,
        })

    # ---------------- run on 8 cores ----------------
    trace = os.environ.get("KERNEL_TRACE", "0") == "1"
    t0 = time.time()
    res = run_bass_kernel_spmd(nc, in_maps, list(range(NCORES)), trace=trace)
    t1 = time.time()
    _last_results["exec_time_ns"] = res.exec_time_ns
    _last_results["wall_s"] = t1 - t0
    _last_results["profile_json"] = res.profile_json

    # ---------------- host-side combine ----------------
    ex_pad = np.empty(total, dtype=np.float32)
    U = np.zeros((NUM_GRAPHS + WIN, H), dtype=np.float64)
    for k in range(NCORES):
        exo = res.results[k]["ex_out"]          # [n_chunks, P, t_chunk]
        ex_pad[k * core_nodes:(k + 1) * core_nodes] = (
            exo.transpose(0, 2, 1).reshape(core_nodes))
        uo = res.results[k]["u_out"]            # [n_chunks, P, H]
        for c in range(n_chunks):
            base = bases[k * n_chunks + c]
            U[base:base + WIN] += uo[c]
    ex = ex_pad[:n]

    if excl_idx.size:
        contrib = ex[excl_idx, None] * x[excl_idx]
        np.add.at(U, batch[excl_idx], contrib)

    Z = np.bincount(batch, weights=ex.astype(np.float64),
                    minlength=NUM_GRAPHS)
    Zsafe = np.where(Z > 0, Z, 1.0)
    pooled = (U[:NUM_GRAPHS] / Zsafe[:, None]).astype(np.float32)
    weights = (ex / Z[batch]).astype(np.float32)
    return pooled, weights
